# revision 27
# baseline (speedup 1.0000x reference)
"""Multi-head attention layer (B=2, L=2048, D=1024, H=16) on 8 trn2 cores.

Sharding: core c handles batch b=c//4 and head group g=c%4 (4 heads, 256 dims).
Each core computes Q/K/V projections for its head-group columns, attention for
its 4 heads, and a partial out-projection (its 256 rows of Wo). The host sums
the 4 partials per batch and adds the bias terms.

Device layout notes:
- Inputs are shipped pre-transposed ([D, L]) and pre-cast to bf16 so every
  matmul contracts over the partition dim without on-device transposes.
- Q/K biases are folded in by augmenting the contraction dim with a ones row
  (row 1024) carrying bq/bk; V bias and bo commute through softmax and are
  added on the host as bv @ Wo + bo.
- Softmax skips the max-subtraction: scores*scale ~ N(0,1), |max| < ~6, safely
  inside fp32/bf16 exp range.
- Scores are computed transposed (S^T: keys on partitions, queries free) so
  exp(S^T) tiles feed the P@V matmul directly as lhsT. The row-sum for the
  softmax denominator comes from a ones column appended to V (M=65), and the
  reciprocal is broadcast across partitions with a K=1 outer-product matmul.
"""
import sys

if "/opt/trn_rl_repo" not in sys.path:
    sys.path.insert(0, "/opt/trn_rl_repo")

import numpy as np
import ml_dtypes

D_MODEL = 1024
N_HEADS = 16
HEAD_DIM = 64
B, L = 2, 2048
N_CORES = 8
GROUPS = 4                      # head groups (tensor-parallel dim)
DG = D_MODEL // GROUPS          # 256 dims per head group
HPG = N_HEADS // GROUPS         # 4 heads per group
DAUG = D_MODEL + 128            # contraction padded with bias row (9*128)
KOQ = DAUG // 128               # 9
KOV = D_MODEL // 128            # 8
KC = L // 128                   # 16 key chunks
QC = 4                          # query chunks of 512
QW = L // QC                    # 512
PAIRS = HPG // 2                # head pairs per core


def _build_program(phases=3, repeat=1, daug=DAUG, mode="full"):
    import os as _os
    NO_FILL = bool(int(_os.environ.get("K_NOFILL", "0")))
    NO_OPFILL = bool(int(_os.environ.get("K_NOOPFILL", "0")))
    LAG_ENV = int(_os.environ.get("K_LAG", "8"))
    NORM_POOL = _os.environ.get("K_NORM", "pool") == "pool"
    SKIP = set(filter(None, _os.environ.get("K_SKIP", "").split(",")))
    CROSS = _os.environ.get("K_CROSS", "0") == "1"
    import concourse.mybir as mybir
    import concourse.tile as tile
    from concourse import bacc

    fp32 = mybir.dt.float32
    bf16 = mybir.dt.bfloat16
    Exp = mybir.ActivationFunctionType.Exp
    KOQ_ = daug // 128

    nc = bacc.Bacc(None, target_bir_lowering=False)

    qT_d = nc.declare_dram_parameter("qT", [daug, L], bf16, isOutput=False)
    kT_d = nc.declare_dram_parameter("kT", [daug, L], bf16, isOutput=False)
    vT_d = nc.declare_dram_parameter("vT", [D_MODEL, L], bf16, isOutput=False)
    wq_d = nc.declare_dram_parameter("wq", [daug, DG], bf16, isOutput=False)
    wk_d = nc.declare_dram_parameter("wk", [daug, DG], bf16, isOutput=False)
    wv_d = nc.declare_dram_parameter("wv", [D_MODEL, DG], bf16, isOutput=False)
    wo_d = nc.declare_dram_parameter("wo", [DG, D_MODEL], bf16, isOutput=False)
    out_d = nc.declare_dram_parameter("out", [L, D_MODEL], fp32, isOutput=True)

    # preamble const: ones row for the K=1 broadcast matmul
    ones_t = nc.alloc_sbuf_tensor("ones_row", [128, 64], bf16)
    nc.gpsimd.memset(ones_t.ap(), 1.0)
    nc.all_engine_barrier()
    ones_ap = ones_t.ap()

    scale = 1.0 / np.sqrt(HEAD_DIM)
    LAG = LAG_ENV  # PV trails S^T/exp by this many key chunks

    with tile.TileContext(nc) as tc:
        def emit_loads(pp, sp):
            # weights double-buffered: their last readers sit deep in the
            # iteration, so single-buffered reloads would serialize the next
            # iteration's lead-in behind them
            wq_sb = pp.tile([128, KOQ_, DG], bf16, tag="wq", bufs=2)
            wk_sb = pp.tile([128, KOQ_, DG], bf16, tag="wk", bufs=2)
            wv_sb = pp.tile([128, KOV, DG], bf16, tag="wvт", bufs=1)
            wo_sb = pp.tile([128, 2, D_MODEL], bf16, tag="woт", bufs=1)
            # startup loads split between the HWDGE lane (sync) and the SWDGE
            # lane (gpsimd) so the lead-in proj operands arrive in parallel;
            # scalar (ACT, idle until the first exp) takes a slice too
            wk_r = wk_d.rearrange("(ko p) n -> p ko n", p=128)
            wq_r = wq_d.rearrange("(ko p) n -> p ko n", p=128)

            qT_sb = sp.tile([128, KOQ_, L], bf16, tag="qT", bufs=1)
            kT_sb = sp.tile([128, KOQ_, L], bf16, tag="kT", bufs=1)
            vT_sb = sp.tile([128, KOV, L], bf16, tag="vT", bufs=1)
            qT_r = qT_d.rearrange("(ko p) f -> p ko f", p=128)
            kT_r = kT_d.rearrange("(ko p) f -> p ko f", p=128)
            vT_r = vT_d.rearrange("(ko p) f -> p ko f", p=128)

            # input DMAs in consumption order: the S^T stream (kT, then the
            # pair-0 qT column) is what keeps ACT fed, so it gets priority;
            # vT (needed by the deferred PV) follows; remaining qT last.
            # The first kT column is striped across issue engines so the
            # lead-in proj_block's operands arrive in parallel.
            nc.sync.dma_start(wk_sb[:], wk_r)
            for ko in range(KOQ_):
                ring = nc.sync if ko % 2 == 0 else nc.gpsimd
                ring.dma_start(kT_sb[:, ko, 0:QW], kT_r[:, ko, 0:QW])
            nc.gpsimd.dma_start(wq_sb[:], wq_r)
            for ko in range(KOQ_):
                ring = nc.sync if ko % 2 == 0 else nc.gpsimd
                ring.dma_start(qT_sb[:, ko, 0:QW], qT_r[:, ko, 0:QW])
            for ko in range(KOV):
                nc.gpsimd.dma_start(vT_sb[:, ko, 0:QW], vT_r[:, ko, 0:QW])
            nc.gpsimd.dma_start(wv_sb[:], wv_d.rearrange("(ko p) n -> p ko n", p=128))
            # bulk prefetch of columns 1-3: one batched transfer per ko
            # (fewer descriptors -> less per-transfer overhead + sem traffic)
            # rest columns split per 512-block so each slice's write only
            # waits its own prior readers and readiness lands incrementally
            for n in range(1, QC):
                nsl = slice(n * QW, (n + 1) * QW)
                for ko in range(KOQ_):
                    ring = nc.sync if ko % 2 == 0 else nc.gpsimd
                    ring.dma_start(kT_sb[:, ko, nsl], kT_r[:, ko, nsl])
                for ko in range(KOV):
                    ring = nc.gpsimd if ko % 2 == 0 else nc.sync
                    ring.dma_start(vT_sb[:, ko, nsl], vT_r[:, ko, nsl])
            for n in range(1, QC):
                nsl = slice(n * QW, (n + 1) * QW)
                for ko in range(KOQ_):
                    ring = nc.gpsimd if ko % 2 == 0 else nc.sync
                    ring.dma_start(qT_sb[:, ko, nsl], qT_r[:, ko, nsl])
            nc.sync.dma_start(wo_sb[:], wo_d.rearrange("(ds p) n -> p ds n", p=128))
            return (wq_sb, wk_sb, wv_sb, wo_sb, qT_sb, kT_sb, vT_sb)

        def emit_leadin(h, wp, pspool, rsb):
            (wq_sb, wk_sb, wv_sb, wo_sb, qT_sb, kT_sb, vT_sb) = h
            QT = wp.tile([128, 2, L], bf16, tag="QTt", bufs=1)
            KT = wp.tile([128, 2, L], bf16, tag="KTt", bufs=1)
            for w_sb, in_sb, dst in ((wk_sb, kT_sb, KT), (wq_sb, qT_sb, QT)):
                for ms in (0,):
                    pa = pspool.tile([128, QW], fp32, tag="porp", bufs=2,
                                     name="pa")
                    for ko in range(KOQ_):
                        nc.tensor.matmul(
                            pa[:], w_sb[:, ko, 0:128], in_sb[:, ko, 0:QW],
                            start=(ko == 0), stop=(ko == KOQ_ - 1),
                            skip_group_check=True,
                        )
                    nc.vector.tensor_copy(dst[:, 0, 0:QW], pa[:])

        def emit_compute(h, wp, ptp, pspool, rsb, out_sb_p,
                         tail_leadin=False):
            (wq_sb, wk_sb, wv_sb, wo_sb, qT_sb, kT_sb, vT_sb) = h
            QT = wp.tile([128, 2, L], bf16, tag="QTt", bufs=1)
            KT = wp.tile([128, 2, L], bf16, tag="KTt", bufs=1)
            Vaug = wp.tile([128, KC, HPG * 65], bf16, tag="Vat", bufs=1)
            OT = wp.tile([128, 2, L], bf16, tag="OTt", bufs=1)

            for h_ in range(HPG):
                nc.vector.memset(Vaug[:, :, 65 * h_ + 64 : 65 * h_ + 65], 1.0)

            lo, hi = slice(0, 64), slice(64, 128)

            # lo+hi psum merge: TensorTensor may read only ONE operand from
            # PSUM (and GpSimd cannot touch PSUM at all), so stage the lo
            # half through SBUF with a DVE copy, then add the hi half on DVE
            def merge(dst, pa_ap, pb_ap, width=QW, view=None):
                stg = rsb.tile([128, QW], fp32, tag="mstg", bufs=3,
                               name="mstg")
                sv = stg[:, 0:width]
                nc.vector.tensor_copy(sv, pa_ap)
                if view is not None:
                    sv = view(sv)
                    pb_ap = view(pb_ap)
                nc.vector.tensor_tensor(dst, sv, pb_ap, mybir.AluOpType.add)

            # ---- block emitters (generators yielding one PE-matmul piece at
            # a time, so filler work drips between attention matmuls).
            # Contractions are split into K-halves accumulated in two separate
            # psum banks with adjacent instructions on disjoint PE row tiles
            # (0,*)/(64,*): the array runs both concurrently (~2x measured),
            # and the lo+hi add is fused into the psum->SBUF copy on DVE.
            def v_block(kc):
                if not CROSS:
                    ps_t = pspool.tile([128, QW], fp32, tag="porp", bufs=2,
                                       name="va")
                    ksl = slice(kc * 128, (kc + 1) * 128)
                    for ko in range(KOV):
                        nc.tensor.matmul(
                            ps_t[:, :DG], vT_sb[:, ko, ksl], wv_sb[:, ko, :],
                            start=(ko == 0), stop=(ko == KOV - 1),
                            skip_group_check=True,
                        )
                        if ko < KOV - 1:
                            yield
                    nc.vector.tensor_copy(
                        Vaug[:, kc, :].rearrange("p (h c) -> p h c", c=65)[:, :, 0:64],
                        ps_t[:, :DG].rearrange("p (h c) -> p h c", c=64),
                    )
                    yield
                    return
                pa = pspool.tile([128, QW], fp32, tag="porp", bufs=2, name="va")
                pb = pspool.tile([128, QW], fp32, tag="porp", bufs=2, name="vb")
                ksl = slice(kc * 128, (kc + 1) * 128)
                for ko in range(KOV):
                    nc.tensor.matmul(
                        pa[:, :DG], vT_sb[lo, ko, ksl], wv_sb[lo, ko, :],
                        start=(ko == 0), stop=(ko == KOV - 1),
                        skip_group_check=True,
                    )
                    yield
                    nc.tensor.matmul(
                        pb[:, :DG], vT_sb[hi, ko, ksl], wv_sb[hi, ko, :],
                        start=(ko == 0), stop=(ko == KOV - 1),
                        skip_group_check=True,
                    )
                    if ko < KOV - 1:
                        yield
                merge(
                    Vaug[:, kc, :].rearrange("p (h c) -> p h c", c=65)[:, :, 0:64],
                    pa[:, :DG], pb[:, :DG], width=DG,
                    view=lambda a: a.rearrange("p (h c) -> p h c", c=64),
                )
                yield

            def proj_block(w_sb, in_sb, dst, ms, nch):
                msl = slice(ms * 128, (ms + 1) * 128)
                nsl = slice(nch * QW, (nch + 1) * QW)
                if not CROSS:
                    ps_t = pspool.tile([128, QW], fp32, tag="porp", bufs=2,
                                       name="pa")
                    for ko in range(KOQ_):
                        nc.tensor.matmul(
                            ps_t[:], w_sb[:, ko, msl], in_sb[:, ko, nsl],
                            start=(ko == 0), stop=(ko == KOQ_ - 1),
                            skip_group_check=True,
                        )
                        if ko < KOQ_ - 1:
                            yield
                    nc.vector.tensor_copy(dst[:, ms, nsl], ps_t[:])
                    yield
                    return
                pa = pspool.tile([128, QW], fp32, tag="porp", bufs=2, name="pa")
                pb = pspool.tile([128, QW], fp32, tag="porp", bufs=2, name="pb")
                for ko in range(KOQ_):
                    nc.tensor.matmul(
                        pa[:], w_sb[lo, ko, msl], in_sb[lo, ko, nsl],
                        start=(ko == 0), stop=(ko == KOQ_ - 1),
                        skip_group_check=True,
                    )
                    yield
                    nc.tensor.matmul(
                        pb[:], w_sb[hi, ko, msl], in_sb[hi, ko, nsl],
                        start=(ko == 0), stop=(ko == KOQ_ - 1),
                        skip_group_check=True,
                    )
                    if ko < KOQ_ - 1:
                        yield
                merge(dst[:, ms, nsl], pa[:], pb[:])
                yield

            def outproj_unit(qt, nch):
                oqsl = slice(qt * 128, (qt + 1) * 128)
                nsl = slice(nch * QW, (nch + 1) * QW)
                if not CROSS:
                    po = pspool.tile([128, QW], fp32, tag="porp", bufs=2,
                                     name="oa")
                    for ds_ in range(2):
                        nc.tensor.matmul(
                            po[:], OT[:, ds_, oqsl], wo_sb[:, ds_, nsl],
                            start=(ds_ == 0), stop=(ds_ == 1),
                            skip_group_check=True,
                        )
                        if ds_ == 0:
                            yield
                    ob = out_sb_p.tile([128, QW], fp32, name="ob")
                    nc.vector.tensor_copy(ob[:], po[:])
                    nc.sync.dma_start(out_d[oqsl, nsl], ob[:])
                    yield
                    return
                pa = pspool.tile([128, QW], fp32, tag="porp", bufs=2, name="oa")
                pb = pspool.tile([128, QW], fp32, tag="porp", bufs=2, name="oc")
                for ds_ in range(2):
                    nc.tensor.matmul(
                        pa[:], OT[lo, ds_, oqsl], wo_sb[lo, ds_, nsl],
                        start=(ds_ == 0), stop=(ds_ == 1),
                        skip_group_check=True,
                    )
                    yield
                    nc.tensor.matmul(
                        pb[:], OT[hi, ds_, oqsl], wo_sb[hi, ds_, nsl],
                        start=(ds_ == 0), stop=(ds_ == 1),
                        skip_group_check=True,
                    )
                    if ds_ == 0:
                        yield
                ob = out_sb_p.tile([128, QW], fp32, name="ob")
                merge(ob[:], pa[:], pb[:])
                nc.sync.dma_start(out_d[oqsl, nsl], ob[:])
                yield

            # filler queue in readiness/deadline order; entries are
            # (piece_ns, n_pieces, gen_maker); emission is budget-paced so
            # filler matmuls drip into the exp-shadow without front-running
            # the score stream on the in-order PE queue.
            PZ, VZ, OZ = (125.0, 62.0, 125.0) if CROSS else (280.0, 145.0, 280.0)
            PP, VP = (2 * KOQ_, 2 * KOV) if CROSS else (KOQ_, KOV)
            OPP = 4 if CROSS else 2

            def K_(ms, n):
                return lambda: proj_block(wk_sb, kT_sb, KT, ms, n)

            def Q_(ms, n):
                return lambda: proj_block(wq_sb, qT_sb, QT, ms, n)

            def V_(kc):
                return lambda: v_block(kc)

            fillers = [
                (("k", 0, 1), PZ, PP, K_(0, 1)),
                (("v", 0), VZ, VP, V_(0)), (("v", 1), VZ, VP, V_(1)),
                (("k", 0, 2), PZ, PP, K_(0, 2)),
                (("v", 2), VZ, VP, V_(2)), (("v", 3), VZ, VP, V_(3)),
                (("k", 0, 3), PZ, PP, K_(0, 3)),
                (("v", 4), VZ, VP, V_(4)), (("v", 5), VZ, VP, V_(5)),
                (("k", 1, 0), PZ, PP, K_(1, 0)),
                (("q", 1, 0), PZ, PP, Q_(1, 0)),
                (("v", 6), VZ, VP, V_(6)), (("v", 7), VZ, VP, V_(7)),
                (("k", 1, 1), PZ, PP, K_(1, 1)),
                (("v", 8), VZ, VP, V_(8)), (("v", 9), VZ, VP, V_(9)),
                (("k", 1, 2), PZ, PP, K_(1, 2)),
                (("v", 10), VZ, VP, V_(10)), (("v", 11), VZ, VP, V_(11)),
                (("k", 1, 3), PZ, PP, K_(1, 3)),
                (("v", 12), VZ, VP, V_(12)), (("v", 13), VZ, VP, V_(13)),
                (("v", 14), VZ, VP, V_(14)), (("v", 15), VZ, VP, V_(15)),
                (("q", 0, 1), PZ, PP, Q_(0, 1)), (("q", 1, 1), PZ, PP, Q_(1, 1)),
                (("q", 0, 2), PZ, PP, Q_(0, 2)), (("q", 1, 2), PZ, PP, Q_(1, 2)),
                (("q", 0, 3), PZ, PP, Q_(0, 3)), (("q", 1, 3), PZ, PP, Q_(1, 3)),
            ]
            filler_pos = [0]              # next unit index in `fillers`
            active_gen = [None, 0, 0.0]   # [generator, remaining, piece_ns]
            emitted = {("k", 0, 0), ("q", 0, 0)}   # lead-in units

            def emit_budget(ns, force_key=None):
                """Advance filler emission by ~ns worth of matmul pieces.
                With force_key, keep emitting whole units (ignoring budget)
                until that unit has been fully emitted."""
                while ns > 0 or (force_key is not None
                                 and force_key not in emitted):
                    if active_gen[0] is None:
                        if filler_pos[0] >= len(fillers):
                            return
                        key, pz, np_, mk = fillers[filler_pos[0]]
                        filler_pos[0] += 1
                        active_gen[0] = mk()
                        active_gen[1] = np_
                        active_gen[2] = pz
                        active_gen.append(key) if len(active_gen) < 4 else None
                        active_gen[3] = key
                    if (active_gen[2] > ns
                            and (force_key is None or force_key in emitted)):
                        return
                    next(active_gen[0])
                    active_gen[1] -= 1
                    ns -= active_gen[2]
                    if active_gen[1] == 0:
                        active_gen[0].close()
                        active_gen[0] = None
                        if active_gen[3] is not None:
                            emitted.add(active_gen[3])

            def require(key):
                """Structural ordering gate: ensure the producer unit for
                `key` has been emitted before its consumer."""
                if key not in emitted:
                    emit_budget(0, force_key=key)

            def emit_all_fillers():
                while filler_pos[0] < len(fillers) or active_gen[0] is not None:
                    emit_budget(1e9)

            if tail_leadin:
                # software-pipelined: this body's K(0,0)/Q(0,0) feed the NEXT
                # loop iteration (the prologue seeded iteration 0), so block 0
                # starts immediately and the lead-in work rides the filler
                # stream
                fillers.append((("k", 0, 0), PZ, PP, K_(0, 0)))
                fillers.append((("q", 0, 0), PZ, PP, Q_(0, 0)))
            else:
                # ---- lead-in: K and Q first columns for pair 0 ----
                for _p in proj_block(wk_sb, kT_sb, KT, 0, 0):
                    pass
                for _p in proj_block(wq_sb, qT_sb, QT, 0, 0):
                    pass
            if NO_FILL:
                emit_all_fillers()

            # ---- attention; pv backlogs carry across block boundaries so
            # the PE never dumps a drain lump into the exp stream ----
            pending_norm = [None]
            carry = []      # pv thunks deferred from the previous block
            op_queue = []   # qc values whose out-projection is ready

            def release_outproj():
                # released one block AFTER the norm emitted, so the PE never
                # reaches an outproj matmul before its OT inputs have long
                # been written (recip->broadcast->scale is a ~3us
                # cross-engine chain)
                while op_queue:
                    qc_r = op_queue.pop(0)
                    for qi in range(QW // 128):
                        qt = qc_r * (QW // 128) + qi
                        for nch in range(2):
                            fillers.append(
                                (None, OZ, OPP,
                                 lambda qt=qt, nch=nch: outproj_unit(qt, nch)))

            def emit_pending_norm():
                if pending_norm[0] is not None:
                    pending_norm[0]()
                    pending_norm[0] = None

            STEP_NS = float(_os.environ.get("K_STEP_NS", "1380"))
            SCORE_NS, PV_NS = 250.0, 560.0
            blocks = [(qc, pair) for qc in range(QC) for pair in range(PAIRS)]
            for bi, (qc, pair) in enumerate(blocks):
                release_outproj()
                qsl = slice(qc * QW, (qc + 1) * QW)
                PT0s, PT1s = {}, {}
                pv0 = pspool.tile([128, QW], fp32, tag="pv0", bufs=1, name="pv0")
                pv1 = pspool.tile([128, QW], fp32, tag="pv1", bufs=1, name="pv1")

                def st_step(kc, pair=pair, qsl=qsl, qc=qc, PT0s=PT0s,
                            PT1s=PT1s):
                    require(("k", pair, kc // 4))
                    require(("q", pair, qc))
                    ksl = slice(kc * 128, (kc + 1) * 128)
                    # both heads' score chunks land in one 2-bank psum tile
                    # (row tiles (0,*)/(64,*) run concurrently on the PE)
                    # so a single exp covers the pair
                    s01 = pspool.tile([128, 2 * QW], fp32, tag="s01",
                                      bufs=2, name="s01")
                    nc.tensor.matmul(
                        s01[:, 0:QW], KT[0:64, pair, ksl],
                        QT[0:64, pair, qsl],
                        start=True, stop=True, skip_group_check=True,
                    )
                    nc.tensor.matmul(
                        s01[:, QW : 2 * QW], KT[64:128, pair, ksl],
                        QT[64:128, pair, qsl],
                        start=True, stop=True, skip_group_check=True,
                    )
                    pt = ptp.tile([128, 2 * QW], bf16, tag="PT", bufs=13,
                                  name="pt")
                    nc.scalar.activation(pt[:], s01[:], Exp, scale=scale)
                    PT0s[kc], PT1s[kc] = pt[:, 0:QW], pt[:, QW : 2 * QW]

                def pv_step(kc, pair=pair, pv0=pv0, pv1=pv1, PT0s=PT0s,
                            PT1s=PT1s):
                    require(("v", kc))
                    h0 = 2 * pair
                    nc.tensor.matmul(
                        pv0[0:65, :],
                        Vaug[:, kc, 65 * h0 : 65 * h0 + 65],
                        PT0s.pop(kc),
                        start=(kc == 0), stop=(kc == KC - 1),
                        skip_group_check=True,
                    )
                    nc.tensor.matmul(
                        pv1[0:65, :],
                        Vaug[:, kc, 65 * (h0 + 1) : 65 * (h0 + 1) + 65],
                        PT1s.pop(kc),
                        start=(kc == 0), stop=(kc == KC - 1),
                        skip_group_check=True,
                    )

                last = bi == len(blocks) - 1
                lag = 4 if last else LAG_ENV
                # block 1 drains late: its predecessor's pv tail needs V
                # chunks that only materialize via block 0/1 fillers
                s_drain = 6 if bi == 1 else 0
                own_next = [0]

                for kc in range(KC):
                    st_step(kc)
                    if "pv" in SKIP:
                        PT0s.pop(kc, None), PT1s.pop(kc, None)
                        emit_budget(STEP_NS - SCORE_NS)
                        continue
                    pe_ns = SCORE_NS
                    if kc >= s_drain:
                        for _ in range(3):
                            if carry:
                                carry.pop(0)()
                                pe_ns += PV_NS
                    if not carry and pending_norm[0] is not None and kc >= 1:
                        emit_pending_norm()
                    if (not carry and pending_norm[0] is None
                            and own_next[0] <= kc - lag):
                        burst = 0
                        while own_next[0] <= kc - lag and burst < 2:
                            pv_step(own_next[0])
                            own_next[0] += 1
                            burst += 1
                            pe_ns += PV_NS
                    emit_budget(STEP_NS - pe_ns)
                # defer the pv tail into the next block's step pacing
                if "pv" not in SKIP:
                    for kc in range(own_next[0], KC):
                        carry.append(lambda kc=kc, pv_step=pv_step: pv_step(kc))

                # softmax denominator: recip (DVE) -> partition broadcast
                # (GpSimd) -> scale (DVE). Out-projection units unlock once
                # both pairs of a qc have normalized, so the closure appends
                # them to the filler queue at emission time.
                def norm(pair=pair, qsl=qsl, pv0=pv0, pv1=pv1, qc=qc,
                         add_op=(pair == PAIRS - 1)):
                    if "norm" in SKIP:
                        if "recip" in SKIP:
                            rcx = rsb.tile([128, 2, QW], fp32, tag="rc",
                                           bufs=2, name="rc")
                            for h01, pv in ((0, pv0), (1, pv1)):
                                nc.vector.reciprocal(rcx[0:1, h01, :],
                                                     pv[64:65, :])
                        for h01, pv in ((0, pv0), (1, pv1)):
                            nc.vector.tensor_copy(
                                OT[64 * h01 : 64 * h01 + 64, pair, qsl],
                                pv[0:64, :])
                        if add_op and not NO_OPFILL and "op" not in SKIP:
                            op_queue.append(qc)
                        return
                    rc = rsb.tile([128, 2, QW], fp32, tag="rc", bufs=1,
                                  name="rc")
                    den = rsb.tile([128, 2, QW], fp32, tag="den", bufs=1,
                                   name="den")
                    for h01, pv in ((0, pv0), (1, pv1)):
                        # ACT stages the psum denominator row into SBUF so the
                        # (slow, single-partition) psum read stays out of the
                        # DVE FIFO that recycles the filler psum banks
                        nc.scalar.copy(den[0:1, h01, :], pv[64:65, :])
                        nc.vector.reciprocal_approx_fast(
                            rc[0:1, h01, :], den[0:1, h01, :])
                    rpsb = rsb.tile([128, QW], fp32, tag="rpsb", bufs=1,
                                    name="rpsb")
                    if NORM_POOL:
                        # the broadcast writes partitions 0..channels of its
                        # destination regardless of AP base, so each head gets
                        # its own base-0 tile
                        rpsb1 = rsb.tile([128, QW], fp32, tag="rpsb1", bufs=1,
                                         name="rpsb1")
                        if "bc" in SKIP:
                            nc.vector.memset(rpsb[0:64, :], 1.0)
                            nc.vector.memset(rpsb1[0:64, :], 1.0)
                        else:
                            nc.gpsimd.partition_broadcast(
                                rpsb[0:64, :], rc[0:1, 0, :], channels=64)
                            nc.gpsimd.partition_broadcast(
                                rpsb1[0:64, :], rc[0:1, 1, :], channels=64)
                        for h01, pv in ((0, pv0), (1, pv1)):
                            src_r = rpsb if h01 == 0 else rpsb1
                            nc.vector.tensor_tensor(
                                OT[64 * h01 : 64 * h01 + 64, pair, qsl],
                                pv[0:64, :],
                                src_r[0:64, :],
                                mybir.AluOpType.mult,
                            )
                        if add_op and not NO_OPFILL and "op" not in SKIP:
                            op_queue.append(qc)
                        return
                    if True:
                        rcb = rsb.tile([128, 2, QW], bf16, tag="rcb", bufs=2,
                                       name="rcb")
                        for h01 in (0, 1):
                            nc.vector.tensor_copy(rcb[0:1, h01, :],
                                                  rc[0:1, h01, :])
                        rp = pspool.tile([128, QW], fp32, tag="porp", bufs=2,
                                         name="rp")
                        nc.tensor.matmul(
                            rp[0:64, :], ones_ap[0:1, :], rcb[0:1, 0, :],
                            start=True, stop=True, skip_group_check=True,
                        )
                        nc.tensor.matmul(
                            rp[64:128, :], ones_ap[0:1, :], rcb[0:1, 1, :],
                            start=True, stop=True, skip_group_check=True,
                        )
                        nc.vector.tensor_copy(rpsb[:], rp[:])
                    for h01, pv in ((0, pv0), (1, pv1)):
                        nc.vector.tensor_tensor(
                            OT[64 * h01 : 64 * h01 + 64, pair, qsl],
                            pv[0:64, :],
                            rpsb[64 * h01 : 64 * h01 + 64, :],
                            mybir.AluOpType.mult,
                        )
                    if add_op and not NO_OPFILL:
                        for qi in range(QW // 128):
                            qt = qc * (QW // 128) + qi
                            for nch in range(2):
                                fillers.append(
                                    (None, OZ, OPP,
                                     lambda qt=qt, nch=nch: outproj_unit(qt, nch)))

                pending_norm[0] = None if "pv" in SKIP else norm

            while carry:
                carry.pop(0)()
            emit_pending_norm()
            release_outproj()
            # drain remaining fillers (final out-projection chunks)
            emit_all_fillers()
            if NO_OPFILL:
                for qt in range(L // 128):
                    for nch in range(2):
                        for _p in outproj_unit(qt, nch):
                            pass

        if mode == "full":
            for _rep in range(repeat):
                with (
                    tc.tile_pool(name="persist", bufs=1) as pp,
                    tc.tile_pool(name="staging", bufs=1) as sp,
                    tc.tile_pool(name="pt", bufs=1) as ptp,
                    tc.tile_pool(name="psum", bufs=1, space="PSUM") as pspool,
                    tc.tile_pool(name="rsb", bufs=1) as rsb,
                    tc.tile_pool(name="out_sb", bufs=4) as out_sb_p,
                ):
                    h = emit_loads(pp, sp)
                    emit_compute(h, pp, ptp, pspool, rsb, out_sb_p)
        elif mode == "compute":
            with (
                tc.tile_pool(name="persist", bufs=1) as pp,
                tc.tile_pool(name="staging", bufs=1) as sp,
            ):
                h = emit_loads(pp, sp)
                for _rep in range(repeat):
                    with (
                        tc.tile_pool(name="work", bufs=1) as wp,
                        tc.tile_pool(name="pt", bufs=1) as ptp,
                        tc.tile_pool(name="psum", bufs=1, space="PSUM") as pspool,
                        tc.tile_pool(name="rsb", bufs=1) as rsb,
                        tc.tile_pool(name="out_sb", bufs=4) as out_sb_p,
                    ):
                        emit_compute(h, wp, ptp, pspool, rsb, out_sb_p)
        elif mode == "dma":
            for _rep in range(repeat):
                with (
                    tc.tile_pool(name="persist", bufs=1) as pp,
                    tc.tile_pool(name="staging", bufs=1) as sp,
                ):
                    emit_loads(pp, sp)
            with tc.tile_pool(name="zout", bufs=1) as zp:
                z = zp.tile([128, QW], fp32)
                nc.vector.memset(z[:], 0.0)
                nc.sync.dma_start(out_d[0:128, 0:QW], z[:])
        elif mode == "loop":
            with (
                tc.tile_pool(name="persist", bufs=1) as pp,
                tc.tile_pool(name="staging", bufs=1) as sp,
                tc.tile_pool(name="pt", bufs=1) as ptp,
                tc.tile_pool(name="psum", bufs=1, space="PSUM") as pspool,
                tc.tile_pool(name="rsb", bufs=1) as rsb,
                tc.tile_pool(name="out_sb", bufs=4) as out_sb_p,
            ):
                with tc.For_i(0, repeat):
                    h = emit_loads(pp, sp)
                    emit_compute(h, pp, ptp, pspool, rsb, out_sb_p)
        elif mode == "dmaloop":
            with (
                tc.tile_pool(name="persist", bufs=1) as pp,
                tc.tile_pool(name="staging", bufs=1) as sp,
                tc.tile_pool(name="zout", bufs=1) as zp,
            ):
                with tc.For_i(0, repeat):
                    emit_loads(pp, sp)
                z = zp.tile([128, QW], fp32)
                nc.vector.memset(z[:], 0.0)
                nc.sync.dma_start(out_d.ap()[0:128, 0:QW], z[:])
        elif mode == "computeloop":
            with (
                tc.tile_pool(name="persist", bufs=1) as pp,
                tc.tile_pool(name="staging", bufs=1) as sp,
                tc.tile_pool(name="pt", bufs=1) as ptp,
                tc.tile_pool(name="psum", bufs=1, space="PSUM") as pspool,
                tc.tile_pool(name="rsb", bufs=1) as rsb,
                tc.tile_pool(name="out_sb", bufs=4) as out_sb_p,
            ):
                h = emit_loads(pp, sp)
                with tc.For_i(0, repeat):
                    emit_compute(h, pp, ptp, pspool, rsb, out_sb_p)
        else:
            raise ValueError(mode)

    nc.compile()
    return nc


class _Runner:
    """Persistent PJRT executable: build/trace once, execute many times.

    Mirrors bass2jax.run_bass_via_pjrt's multi-core shard_map path, but keeps
    the jitted callable (and the NEFF) alive across calls and skips output
    donation — this kernel writes every output element, so pre-zeroed output
    buffers aren't needed.
    """

    def __init__(self, build_fn=None):
        import jax
        import numpy as _np
        from jax.sharding import Mesh, PartitionSpec
        from jax.experimental.shard_map import shard_map
        import concourse.mybir as mybir
        from concourse import bass2jax

        bass2jax.install_neuronx_cc_hook()
        self.nc = nc = (build_fn or _build_program)()
        self.jax = jax

        partition_name = (
            nc.partition_id_tensor.name if nc.partition_id_tensor else None
        )
        in_names, out_names, out_avals = [], [], []
        for alloc in nc.m.functions[0].allocations:
            if not isinstance(alloc, mybir.MemoryLocationSet):
                continue
            name = alloc.memorylocations[0].name
            if alloc.kind == "ExternalInput":
                if name != partition_name:
                    in_names.append(name)
            elif alloc.kind == "ExternalOutput":
                out_names.append(name)
                out_avals.append(
                    jax.core.ShapedArray(
                        tuple(alloc.tensor_shape), mybir.dt.np(alloc.dtype)
                    )
                )
        self.in_names, self.out_names, self.out_avals = in_names, out_names, out_avals
        n_params = len(in_names)
        zero_outs = [
            _np.zeros((N_CORES * a.shape[0], *a.shape[1:]), a.dtype) for a in out_avals
        ]

        body_in_names = in_names + out_names
        if partition_name is not None:
            body_in_names = body_in_names + [partition_name]

        def _body(*args):
            operands = list(args)
            if partition_name is not None:
                operands.append(bass2jax.partition_id_tensor())
            outs = bass2jax._bass_exec_p.bind(
                *operands,
                out_avals=tuple(out_avals),
                in_names=tuple(body_in_names),
                out_names=tuple(out_names),
                lowering_input_output_aliases=(),
                sim_require_finite=True,
                sim_require_nnan=True,
                nc=nc,
            )
            return tuple(outs)

        self._body = _body
        devices = jax.devices()[:N_CORES]
        self.mesh = Mesh(_np.asarray(devices), ("core",))
        in_specs = (PartitionSpec("core"),) * (n_params + len(out_names))
        out_specs = (PartitionSpec("core"),) * len(out_names)
        self.fn = jax.jit(
            shard_map(
                _body,
                mesh=self.mesh,
                in_specs=in_specs,
                out_specs=out_specs,
                check_rep=False,
            ),
            keep_unused=True,
        )
        self.sharding = jax.sharding.NamedSharding(self.mesh, PartitionSpec("core"))
        self.zeros_dev = [jax.device_put(z, self.sharding) for z in zero_outs]

    def make_chained(self, k):
        """Jitted fn executing the NEFF k times back-to-back (output buffers
        threaded into the next call), for marginal-cost timing."""
        import numpy as _np
        from jax.sharding import PartitionSpec
        from jax.experimental.shard_map import shard_map

        n_params = len(self.in_names)
        body = self._body

        def _chain(*args):
            ins, outs = args[:n_params], args[n_params:]
            for _ in range(k):
                outs = body(*ins, *outs)
            return outs

        in_specs = (PartitionSpec("core"),) * (n_params + len(self.out_names))
        out_specs = (PartitionSpec("core"),) * len(self.out_names)
        return self.jax.jit(
            shard_map(
                _chain,
                mesh=self.mesh,
                in_specs=in_specs,
                out_specs=out_specs,
                check_rep=False,
            ),
            keep_unused=True,
        )

    def put_inputs(self, in_maps):
        import numpy as _np

        concat = [
            _np.concatenate([m[name] for m in in_maps], axis=0)
            for name in self.in_names
        ]
        return [self.jax.device_put(c, self.sharding) for c in concat]

    def execute(self, dev_inputs):
        outs = self.fn(*dev_inputs, *self.zeros_dev)
        self.jax.block_until_ready(outs)
        return outs

    def run(self, in_maps):
        import numpy as _np

        outs = self.execute(self.put_inputs(in_maps))
        return [
            {
                name: _np.asarray(outs[i]).reshape(
                    N_CORES, *self.out_avals[i].shape
                )[c]
                for i, name in enumerate(self.out_names)
            }
            for c in range(N_CORES)
        ]


_RUNNERS = {}


def _get_runner(daug=DAUG):
    if daug not in _RUNNERS:
        _RUNNERS[daug] = _Runner(lambda: _build_program(daug=daug))
    return _RUNNERS[daug]


def kernel(query, key, value, Wq, bq, Wk, bk, Wv, bv, Wo, bo, **extra):

    bf = ml_dtypes.bfloat16
    query = np.asarray(query, np.float32)
    key = np.asarray(key, np.float32)
    value = np.asarray(value, np.float32)
    Wq = np.asarray(Wq, np.float32)
    Wk = np.asarray(Wk, np.float32)
    Wv = np.asarray(Wv, np.float32)
    Wo = np.asarray(Wo, np.float32)
    bq = np.asarray(bq, np.float32)
    bk = np.asarray(bk, np.float32)
    bv = np.asarray(bv, np.float32)
    bo = np.asarray(bo, np.float32)

    # zero q/k biases (the generated case) skip the bias-augmented row
    daug = D_MODEL if (not bq.any() and not bk.any()) else DAUG
    runner = _get_runner(daug)

    # host-side shard prep: transpose + bias-augment + cast
    def aug_T(x):  # [L, D] -> [daug, L] (ones row at 1024 when augmented)
        if daug == D_MODEL:
            return np.ascontiguousarray(x.T).astype(bf)
        xa = np.zeros((daug, L), np.float32)
        xa[:D_MODEL] = x.T
        xa[D_MODEL] = 1.0
        return xa.astype(bf)

    def aug_W(w, b):  # [D, DG-slice] (+ bias row when augmented)
        if daug == D_MODEL:
            return np.ascontiguousarray(w).astype(bf)
        wa = np.zeros((daug, w.shape[1]), np.float32)
        wa[:D_MODEL] = w
        wa[D_MODEL] = b
        return wa.astype(bf)

    qTs = [aug_T(query[b]) for b in range(B)]
    kTs = [aug_T(key[b]) for b in range(B)]
    vTs = [value[b].T.astype(bf) for b in range(B)]

    in_maps = []
    for c in range(N_CORES):
        b, g = divmod(c, GROUPS)
        gs = slice(g * DG, (g + 1) * DG)
        in_maps.append({
            "qT": qTs[b],
            "kT": kTs[b],
            "vT": vTs[b],
            "wq": aug_W(Wq[:, gs], bq[gs]),
            "wk": aug_W(Wk[:, gs], bk[gs]),
            "wv": Wv[:, gs].astype(bf),
            "wo": Wo[gs, :].astype(bf),
        })

    global _LAST_IN_MAPS
    _LAST_IN_MAPS = in_maps
    results = runner.run(in_maps)

    host_bias = (bv.astype(np.float32) @ Wo.astype(np.float32)) + bo
    out = np.zeros((B, L, D_MODEL), np.float32)
    for c in range(N_CORES):
        b = c // GROUPS
        out[b] += results[c]["out"]
    out += host_bias
    return out

